# revision 13
# baseline (speedup 1.0000x reference)
"""AttnDecoderRNN step on 8 TRN2 NeuronCores (v5).

Sharding (tensor-parallel over the fat dims):
  - out_W [V=50257, H]: vocab-sharded 8 x 6400 (padded to 51200), f32,
    streamed over SWDGE and split between PE (transposed tiles, GEMV via
    matmul) and DVE (natural tiles, multiply+reduce) so both engines
    consume the stream in parallel.
  - comb_W: output-row sharded -> x slice computed locally in column form,
    AllGather x (4KB).
  - w_ih/w_hh: gate-aligned output-row shards (384 rows each) -> local
    h_new slice, AllGather h_new (4KB).
  - attention: replicated (tiny).
  - emb: token row gathered host-side (4KB of a 206MB table).
Collectives (all AllGather/bypass): x (4KB), h_new (4KB), log-softmax
stats (64B).  log_softmax normalization is computed on device with the
cross-device max/sum.
"""

from contextlib import ExitStack

import numpy as np

import concourse.bacc as bacc
import concourse.mybir as mybir
import concourse.tile as tile
from concourse.bass_utils import run_bass_kernel_spmd
from concourse.masks import make_identity

H = 1024
V = 50257
ML = 30
NCORES = 8
VS = 6400              # per-core vocab shard (8*6400 = 51200 >= V)
NVT = VS // 128        # 50 v-tiles per core
PE_T = 50              # v-tiles handled by PE (transposed layout)
DVE_T = NVT - PE_T     # v-tiles handled by DVE (natural layout)
PE_CH = 10             # PE stream chunks
PE_TPC = PE_T // PE_CH         # 5 tiles per PE chunk
DVE_G = 1              # DVE stream groups (unused when DVE_T=0)
DVE_TPG = DVE_T // DVE_G       # 5 tiles per DVE group
HT = H // 128          # 8 k-tiles over the hidden dim
F32 = mybir.dt.float32
AX = mybir.AxisListType.X
AF = mybir.ActivationFunctionType
ALU = mybir.AluOpType
RG = [list(range(NCORES))]

_CACHE = {}


def _build_program():
    nc = bacc.Bacc("TRN2", target_bir_lowering=False, debug=False, num_devices=NCORES)

    # ---- per-core external inputs (host pre-sliced / pre-transposed) ----
    emb_row = nc.dram_tensor("emb_row", [H], F32, kind="ExternalInput")
    h0 = nc.dram_tensor("h0", [H], F32, kind="ExternalInput")
    h0p = nc.dram_tensor("h0p", [128], F32, kind="ExternalInput")
    enc = nc.dram_tensor("enc", [ML, H], F32, kind="ExternalInput")
    attn_wt = nc.dram_tensor("attn_wt", [2 * H, ML], F32, kind="ExternalInput")
    attn_b = nc.dram_tensor("attn_b", [ML], F32, kind="ExternalInput")
    comb_wt = nc.dram_tensor("comb_wt", [2 * H, 128], F32, kind="ExternalInput")
    comb_bc = nc.dram_tensor("comb_bc", [128], F32, kind="ExternalInput")
    w_iht = nc.dram_tensor("w_iht", [H, 384], F32, kind="ExternalInput")
    w_hht = nc.dram_tensor("w_hht", [H, 384], F32, kind="ExternalInput")
    b_ih = nc.dram_tensor("b_ih", [384], F32, kind="ExternalInput")
    b_hh = nc.dram_tensor("b_hh", [384], F32, kind="ExternalInput")
    out_wt_pe = nc.dram_tensor("out_wt_pe", [H, PE_T * 128], F32, kind="ExternalInput")
    out_w_dve = nc.dram_tensor("out_w_dve", [max(DVE_T, 1) * 128, H], F32, kind="ExternalInput")
    out_bc = nc.dram_tensor("out_bc", [VS], F32, kind="ExternalInput")

    # ---- per-core external outputs ----
    # logits_out[p, t] = log_softmax(logits)[core_base + t*128 + p]
    logits_out = nc.dram_tensor("logits_out", [128, NVT], F32, kind="ExternalOutput")
    hnew_out = nc.dram_tensor("hnew_out", [H], F32, kind="ExternalOutput")
    attnw_out = nc.dram_tensor("attnw_out", [1, ML], F32, kind="ExternalOutput")

    # ---- internal DRAM for collectives ----
    x_in = nc.dram_tensor("x_in", [128], F32)
    x_out = nc.dram_tensor("x_out", [H], F32, addr_space="Shared")
    hn_in = nc.dram_tensor("hn_in", [128], F32)
    hn_out = nc.dram_tensor("hn_out", [H], F32, addr_space="Shared")
    st_in = nc.dram_tensor("st_in", [2], F32)
    st_out = nc.dram_tensor("st_out", [2 * NCORES], F32, addr_space="Shared")

    with tile.TileContext(nc) as tc, ExitStack() as ctx:
        sb = ctx.enter_context(tc.tile_pool(name="sb", bufs=1))
        ps = ctx.enter_context(tc.tile_pool(name="ps", bufs=2, space="PSUM"))
        ps_mm = ctx.enter_context(tc.tile_pool(name="ps_mm", bufs=4, space="PSUM"))
        wpool = ctx.enter_context(tc.tile_pool(name="wpool", bufs=4))
        dpool = ctx.enter_context(tc.tile_pool(name="dpool", bufs=2))

        # ---------- small loads first (these gate the front chain) ----------
        cat1 = sb.tile([128, 2 * HT], F32)  # cols 0-7 embedded, 8-15 h0 (col layout)
        nc.sync.dma_start(out=cat1[:, 0:HT], in_=emb_row[:].rearrange("(f p) -> p f", p=128))
        nc.sync.dma_start(out=cat1[:, HT:2 * HT], in_=h0[:].rearrange("(f p) -> p f", p=128))
        h0p_row = sb.tile([1, 128], F32)
        nc.sync.dma_start(out=h0p_row[:], in_=h0p[:].rearrange("(o n) -> o n", o=1))
        comb_bc_col = sb.tile([128, 1], F32)
        nc.sync.dma_start(out=comb_bc_col[:], in_=comb_bc[:].rearrange("(p o) -> p o", o=1))
        attnb_sb = sb.tile([1, ML], F32)
        nc.sync.dma_start(out=attnb_sb[:], in_=attn_b[:].rearrange("(o j) -> o j", o=1))
        b_ih_sb = sb.tile([1, 384], F32)
        nc.sync.dma_start(out=b_ih_sb[:], in_=b_ih[:].rearrange("(o n) -> o n", o=1))
        b_hh_sb = sb.tile([1, 384], F32)
        nc.sync.dma_start(out=b_hh_sb[:], in_=b_hh[:].rearrange("(o n) -> o n", o=1))
        attnwt_sb = sb.tile([128, 2 * HT, ML], F32)
        nc.sync.dma_start(out=attnwt_sb[:], in_=attn_wt[:].rearrange("(kt p) j -> p kt j", p=128))
        enc_pad = sb.tile([128, H], F32)  # K-padded to 128 partitions
        nc.vector.memset(enc_pad[:], 0.0)
        nc.sync.dma_start(out=enc_pad[0:ML, :], in_=enc[:])
        outb_pe_col = sb.tile([128, PE_T], F32)
        nc.sync.dma_start(out=outb_pe_col[:],
                          in_=out_bc[0:PE_T * 128].rearrange("(f p) -> p f", p=128))
        if DVE_T:
            outb_dve_col = sb.tile([128, DVE_T], F32)
            nc.sync.dma_start(out=outb_dve_col[:],
                              in_=out_bc[PE_T * 128:VS].rearrange("(f p) -> p f", p=128))

        comb_wt_sb = sb.tile([128, 2 * HT, 128], F32)
        nc.sync.dma_start(out=comb_wt_sb[:], in_=comb_wt[:].rearrange("(kt p) n -> p kt n", p=128))
        w_hht_sb = sb.tile([128, HT, 384], F32)
        nc.sync.dma_start(out=w_hht_sb[:], in_=w_hht[:].rearrange("(ht p) n -> p ht n", p=128))
        w_iht_sb = sb.tile([128, HT, 384], F32)
        nc.sync.dma_start(out=w_iht_sb[:], in_=w_iht[:].rearrange("(ht p) n -> p ht n", p=128))

        ident = sb.tile([128, 128], F32)
        make_identity(nc, ident[:])

        # ---------- big out_W streams on SWDGE (gpsimd) queues ----------
        out_wt_r = out_wt_pe[:].rearrange("(ht p) v -> p ht v", p=128)
        out_w_d = out_w_dve[:].rearrange("(t p) h -> p t h", p=128)
        pe_chunks = []
        for ci in range(PE_CH):
            wt_t = wpool.tile([128, HT, PE_TPC * 128], F32, tag="pe")
            nc.sync.dma_start(out=wt_t[:],
                              in_=out_wt_r[:, :, ci * PE_TPC * 128:(ci + 1) * PE_TPC * 128])
            pe_chunks.append(wt_t)
        dve_groups = []
        if DVE_T:
            for gi_ in range(DVE_G):
                wd_t = dpool.tile([128, DVE_TPG, H], F32, tag="dve")
                nc.sync.dma_start(out=wd_t[:],
                                  in_=out_w_d[:, gi_ * DVE_TPG:(gi_ + 1) * DVE_TPG, :])
                dve_groups.append(wd_t)

        # ---------- attention ----------
        psc = ps.tile([1, ML], F32, tag="fr")
        for kt in range(2 * HT):
            nc.tensor.matmul(psc[:], lhsT=cat1[:, kt:kt + 1], rhs=attnwt_sb[:, kt, :],
                             start=(kt == 0), stop=(kt == 2 * HT - 1))
        scores = sb.tile([1, ML], F32)
        nc.vector.tensor_add(scores[:], psc[:], attnb_sb[:])
        mx = sb.tile([1, 1], F32)
        nc.vector.reduce_max(mx[:], scores[:], axis=AX)
        negmx = sb.tile([1, 1], F32)
        nc.vector.tensor_scalar_mul(negmx[:], mx[:], -1.0)
        esc = sb.tile([1, ML], F32)
        ssum = sb.tile([1, 1], F32)
        nc.scalar.activation(esc[:], scores[:], AF.Exp, bias=negmx[:], scale=1.0,
                             accum_out=ssum[:])
        rs = sb.tile([1, 1], F32)
        nc.vector.reciprocal(rs[:], ssum[:])
        attnw = sb.tile([1, ML], F32)
        nc.vector.tensor_scalar_mul(attnw[:], esc[:], rs[:])
        nc.gpsimd.dma_start(out=attnw_out[:], in_=attnw[:])

        # transpose attnw [1,30] -> [30,1] via K-padded PE matmul
        attnw_pad = sb.tile([128, ML], F32)
        nc.vector.memset(attnw_pad[:], 0.0)
        nc.vector.tensor_copy(attnw_pad[0:1, :], attnw[:])
        e0 = sb.tile([128, 1], F32)
        nc.vector.memset(e0[:], 0.0)
        nc.vector.memset(e0[0:1, :], 1.0)
        pw = ps.tile([ML, 1], F32, tag="fr")
        nc.tensor.matmul(pw[:], lhsT=attnw_pad[:], rhs=e0[:], start=True, stop=True)
        wcol_pad = sb.tile([128, 1], F32)
        nc.vector.memset(wcol_pad[:], 0.0)
        nc.vector.tensor_copy(wcol_pad[0:ML, :], pw[:])

        # attn_applied into cat2 cols 8-15; cols 0-7 = embedded
        cat2 = sb.tile([128, 2 * HT], F32)
        nc.vector.tensor_copy(cat2[:, 0:HT], cat1[:, 0:HT])
        for ht in range(HT):
            paa = ps_mm.tile([128, 1], F32, tag="mm")
            nc.tensor.matmul(paa[:], lhsT=enc_pad[:, ht * 128:(ht + 1) * 128],
                             rhs=wcol_pad[:], start=True, stop=True)
            nc.vector.tensor_copy(cat2[:, HT + ht:HT + ht + 1], paa[:])

        # ---------- combine: x slice in column form, then AllGather ----------
        pxc = ps.tile([128, 1], F32, tag="fr")
        for kt in range(2 * HT):
            nc.tensor.matmul(pxc[:], lhsT=comb_wt_sb[:, kt, :], rhs=cat2[:, kt:kt + 1],
                             start=(kt == 0), stop=(kt == 2 * HT - 1))
        xb_col = sb.tile([128, 1], F32)
        nc.vector.tensor_add(xb_col[:], pxc[:], comb_bc_col[:])
        x_col = sb.tile([128, 1], F32)
        nc.scalar.activation(x_col[:], xb_col[:], AF.Relu)
        nc.gpsimd.dma_start(out=x_in[:].rearrange("(p o) -> p o", o=1), in_=x_col[:])
        nc.gpsimd.collective_compute("AllGather", ALU.bypass,
                                     replica_groups=RG, ins=[x_in[:]], outs=[x_out[:]])
        x_colf = sb.tile([128, HT], F32)
        nc.gpsimd.dma_start(out=x_colf[:], in_=x_out[:].rearrange("(f p) -> p f", p=128))

        # ---------- GRU (gate-aligned output-row shard) ----------
        pgi = ps.tile([1, 384], F32, tag="fr")
        for ht in range(HT):
            nc.tensor.matmul(pgi[:], lhsT=x_colf[:, ht:ht + 1], rhs=w_iht_sb[:, ht, :],
                             start=(ht == 0), stop=(ht == HT - 1))
        pgh = ps.tile([1, 384], F32, tag="fr")
        for ht in range(HT):
            nc.tensor.matmul(pgh[:], lhsT=cat1[:, HT + ht:HT + ht + 1], rhs=w_hht_sb[:, ht, :],
                             start=(ht == 0), stop=(ht == HT - 1))
        gi = sb.tile([1, 384], F32)
        nc.vector.tensor_add(gi[:], pgi[:], b_ih_sb[:])
        gh = sb.tile([1, 384], F32)
        nc.vector.tensor_add(gh[:], pgh[:], b_hh_sb[:])
        rpre = sb.tile([1, 128], F32)
        nc.vector.tensor_add(rpre[:], gi[:, 0:128], gh[:, 0:128])
        r = sb.tile([1, 128], F32)
        nc.scalar.activation(r[:], rpre[:], AF.Sigmoid)
        zpre = sb.tile([1, 128], F32)
        nc.vector.tensor_add(zpre[:], gi[:, 128:256], gh[:, 128:256])
        z = sb.tile([1, 128], F32)
        nc.scalar.activation(z[:], zpre[:], AF.Sigmoid)
        t1 = sb.tile([1, 128], F32)
        nc.vector.tensor_mul(t1[:], r[:], gh[:, 256:384])
        npre = sb.tile([1, 128], F32)
        nc.vector.tensor_add(npre[:], t1[:], gi[:, 256:384])
        ng = sb.tile([1, 128], F32)
        nc.scalar.activation(ng[:], npre[:], AF.Tanh)
        d = sb.tile([1, 128], F32)
        nc.vector.tensor_sub(d[:], h0p_row[:], ng[:])
        t3 = sb.tile([1, 128], F32)
        nc.vector.tensor_mul(t3[:], z[:], d[:])
        hnp = sb.tile([1, 128], F32)
        nc.vector.tensor_add(hnp[:], ng[:], t3[:])
        nc.gpsimd.dma_start(out=hn_in[:].rearrange("(o n) -> o n", o=1), in_=hnp[:])
        nc.gpsimd.collective_compute("AllGather", ALU.bypass,
                                     replica_groups=RG, ins=[hn_in[:]], outs=[hn_out[:]])

        hn_col = sb.tile([128, HT], F32)
        nc.gpsimd.dma_start(out=hn_col[:], in_=hn_out[:].rearrange("(f p) -> p f", p=128))
        if DVE_T:
            hb = sb.tile([128, H], F32)
            nc.gpsimd.dma_start(out=hb[:], in_=hn_out[:].rearrange("(o n) -> o n", o=1)
                                .to_broadcast((128, H)))
        nc.gpsimd.dma_start(out=hnew_out[:], in_=hn_out[:])

        # ---------- GEMV: PE half (transposed tiles) ----------
        logits_sb = sb.tile([128, NVT], F32)
        for ci in range(PE_CH):
            wt_t = pe_chunks[ci]
            for lt in range(PE_TPC):
                vt = ci * PE_TPC + lt
                pl = ps_mm.tile([128, 1], F32, tag="mm")
                for ht in range(HT):
                    nc.tensor.matmul(pl[:], lhsT=wt_t[:, ht, lt * 128:(lt + 1) * 128],
                                     rhs=hn_col[:, ht:ht + 1],
                                     start=(ht == 0), stop=(ht == HT - 1))
                nc.vector.tensor_scalar_add(logits_sb[:, vt:vt + 1], pl[:],
                                            outb_pe_col[:, vt:vt + 1])

        # ---------- GEMV: DVE half (natural tiles, multiply+reduce) ----------
        if DVE_T:
            for gi_ in range(DVE_G):
                wd_t = dve_groups[gi_]
                for lt in range(DVE_TPG):
                    vt = PE_T + gi_ * DVE_TPG + lt
                    junk = dpool.tile([128, H], F32, tag="junk")
                    nc.vector.tensor_tensor_reduce(
                        out=junk[:], in0=wd_t[:, lt, :], in1=hb[:],
                        scale=1.0, scalar=outb_dve_col[:, vt - PE_T:vt - PE_T + 1],
                        op0=ALU.mult, op1=ALU.add,
                        accum_out=logits_sb[:, vt:vt + 1])

        # ---------- local log-softmax stats ----------
        mp = sb.tile([128, 1], F32)
        nc.vector.reduce_max(mp[:], logits_sb[:], axis=AX)
        negmp = sb.tile([128, 1], F32)
        nc.vector.tensor_scalar_mul(negmp[:], mp[:], -1.0)
        e_sb = sb.tile([128, NVT], F32)
        sp = sb.tile([128, 1], F32)
        nc.scalar.activation(e_sb[:], logits_sb[:], AF.Exp, bias=negmp[:], scale=1.0,
                             accum_out=sp[:])
        pTm = ps.tile([1, 128], F32, tag="fr")
        nc.tensor.transpose(pTm[:], mp[:], ident[:])
        pTs = ps.tile([1, 128], F32, tag="fr")
        nc.tensor.transpose(pTs[:], sp[:], ident[:])
        mrow = sb.tile([1, 128], F32)
        nc.vector.tensor_copy(mrow[:], pTm[:])
        srow = sb.tile([1, 128], F32)
        nc.vector.tensor_copy(srow[:], pTs[:])
        mloc = sb.tile([1, 1], F32)
        nc.vector.reduce_max(mloc[:], mrow[:], axis=AX)
        negml = sb.tile([1, 1], F32)
        nc.vector.tensor_scalar_mul(negml[:], mloc[:], -1.0)
        erow = sb.tile([1, 128], F32)
        nc.scalar.activation(erow[:], mrow[:], AF.Exp, bias=negml[:], scale=1.0)
        trow = sb.tile([1, 128], F32)
        nc.vector.tensor_mul(trow[:], erow[:], srow[:])
        sloc = sb.tile([1, 1], F32)
        nc.vector.reduce_sum(sloc[:], trow[:], axis=AX)

        nc.gpsimd.dma_start(out=st_in[0:1].rearrange("(o n) -> o n", o=1), in_=mloc[:])
        nc.gpsimd.dma_start(out=st_in[1:2].rearrange("(o n) -> o n", o=1), in_=sloc[:])
        nc.gpsimd.collective_compute("AllGather", ALU.bypass,
                                     replica_groups=RG, ins=[st_in[:]], outs=[st_out[:]])
        sg = sb.tile([1, NCORES, 2], F32)
        nc.gpsimd.dma_start(out=sg[:], in_=st_out[:].rearrange("(o c t) -> o c t", o=1, t=2))

        # logZ = M + ln(sum_c s_c * exp(m_c - M))
        Mg = sb.tile([1, 1], F32)
        nc.vector.reduce_max(Mg[:], sg[:, :, 0], axis=AX)
        negMg = sb.tile([1, 1], F32)
        nc.vector.tensor_scalar_mul(negMg[:], Mg[:], -1.0)
        e8 = sb.tile([1, NCORES], F32)
        nc.scalar.activation(e8[:], sg[:, :, 0], AF.Exp, bias=negMg[:], scale=1.0)
        t8 = sb.tile([1, NCORES], F32)
        nc.vector.tensor_mul(t8[:], e8[:], sg[:, :, 1])
        Sg = sb.tile([1, 1], F32)
        nc.vector.reduce_sum(Sg[:], t8[:], axis=AX)
        lnS = sb.tile([1, 1], F32)
        nc.scalar.activation(lnS[:], Sg[:], AF.Ln)
        neglz = sb.tile([1, 1], F32)
        nc.vector.tensor_add(neglz[:], Mg[:], lnS[:])
        nc.vector.tensor_scalar_mul(neglz[:], neglz[:], -1.0)

        # broadcast -logZ to [128,1] via padded-column PE matmul, subtract, store
        ones_full = sb.tile([128, 128], F32)
        nc.vector.memset(ones_full[:], 1.0)
        padcol = sb.tile([128, 1], F32)
        nc.vector.memset(padcol[:], 0.0)
        nc.vector.tensor_copy(padcol[0:1, :], neglz[:])
        pbz = ps.tile([128, 1], F32, tag="fr")
        nc.tensor.matmul(pbz[:], lhsT=ones_full[:], rhs=padcol[:], start=True, stop=True)
        negz_col = sb.tile([128, 1], F32)
        nc.vector.tensor_copy(negz_col[:], pbz[:])
        final = sb.tile([128, NVT], F32)
        nc.vector.tensor_scalar_add(final[:], logits_sb[:], negz_col[:])
        nc.gpsimd.dma_start(out=logits_out[:], in_=final[:])

    nc.finalize()
    return nc


def _get_nc():
    if "nc" not in _CACHE:
        _CACHE["nc"] = _build_program()
    return _CACHE["nc"]


def _prep_in_maps(inputs):
    token = int(np.asarray(inputs["token"]).reshape(-1)[0])
    emb = np.asarray(inputs["emb"], dtype=np.float32)
    emb_row = np.ascontiguousarray(emb[token])
    h0 = np.ascontiguousarray(np.asarray(inputs["hidden"], np.float32).reshape(H))
    enc = np.ascontiguousarray(np.asarray(inputs["encoder_outputs"], np.float32))
    attn_wt = np.ascontiguousarray(np.asarray(inputs["attn_W"], np.float32).T)
    attn_b = np.ascontiguousarray(np.asarray(inputs["attn_b"], np.float32))
    comb_wt_full = np.asarray(inputs["comb_W"], np.float32).T  # [2H, H]
    comb_b = np.asarray(inputs["comb_b"], np.float32)
    w_ih = np.asarray(inputs["w_ih"], np.float32)
    w_hh = np.asarray(inputs["w_hh"], np.float32)
    b_ih = np.asarray(inputs["b_ih"], np.float32)
    b_hh = np.asarray(inputs["b_hh"], np.float32)
    out_w = np.asarray(inputs["out_W"], np.float32)
    out_b = np.asarray(inputs["out_b"], np.float32)

    out_w_pad = np.zeros((NCORES * VS, H), np.float32)
    out_w_pad[:V] = out_w
    out_b_pad = np.full(NCORES * VS, -1e30, np.float32)
    out_b_pad[:V] = out_b

    in_maps = []
    for c in range(NCORES):
        hs = slice(c * 128, (c + 1) * 128)
        gidx = np.concatenate([np.arange(g * H + c * 128, g * H + (c + 1) * 128)
                               for g in range(3)])
        shard = out_w_pad[c * VS:(c + 1) * VS]  # [6400, H]
        in_maps.append({
            "emb_row": emb_row,
            "h0": h0,
            "h0p": np.ascontiguousarray(h0[hs]),
            "enc": enc,
            "attn_wt": attn_wt,
            "attn_b": attn_b,
            "comb_wt": np.ascontiguousarray(comb_wt_full[:, hs]),
            "comb_bc": np.ascontiguousarray(comb_b[hs]),
            "w_iht": np.ascontiguousarray(w_ih[gidx].T),   # [H, 384]
            "w_hht": np.ascontiguousarray(w_hh[gidx].T),   # [H, 384]
            "b_ih": np.ascontiguousarray(b_ih[gidx]),
            "b_hh": np.ascontiguousarray(b_hh[gidx]),
            "out_wt_pe": np.ascontiguousarray(shard[:PE_T * 128].T),   # [H, 2560]
            "out_w_dve": np.ascontiguousarray(shard[PE_T * 128:] if DVE_T else shard[:128]),
            "out_bc": np.ascontiguousarray(out_b_pad[c * VS:(c + 1) * VS]),
        })
    return in_maps


def _assemble(results):
    parts = [np.asarray(results[c]["logits_out"]).T.reshape(VS) for c in range(NCORES)]
    output = np.concatenate(parts)[:V].astype(np.float32)
    new_hidden = np.asarray(results[0]["hnew_out"], np.float32).reshape(1, 1, H)
    attn_weights = np.asarray(results[0]["attnw_out"], np.float32).reshape(1, ML)
    return output, new_hidden, attn_weights


def run(inputs, trace=False):
    nc = _get_nc()
    in_maps = _prep_in_maps(inputs)
    br = run_bass_kernel_spmd(nc, in_maps, list(range(NCORES)), trace=trace)
    return _assemble(br.results), br


def kernel(**inputs):
    (output, new_hidden, attn_weights), _ = run(inputs)
    return output, new_hidden, attn_weights
